# revision 8
# baseline (speedup 1.0000x reference)
"""CrossAttentionBlock Trainium2 kernel (v2).

Shapes (hardcoded): x (16, 512, 64, 64) f32, context (16, 77, 768) f32.
Sharding: data-parallel over batch B=16 across 8 cores (2 batches/core).

Key structure (vs the naive per-op mapping):
  - All normalization affines and biases are folded into weights/biases:
      * LayerNorm w/b folded into k/v weights+biases on HOST.
      * v-bias + out-bias folded into a per-channel constant cb = out_b +
        out_w @ (v_b + v_w @ ln_b), pre-added to x on HOST (x ships as bf16
        with cb added; device subtracts cb from the measured channel means).
      * GroupNorm apply folded into the q projection per batch on DEVICE:
        qwT_s[c,o] = qwT[c,o]*scale[c] (4 small DVE ops) and the shift term
        becomes a per-output-channel bias via 16 tiny matmuls, applied for
        free in the qT psum evacuation.
      * Residual add folded into the out-projection accumulation via an
        identity-matrix matmul; the psum evacuation is a single-src copy
        that the Tile scheduler can place on either ACT or DVE.
  - Softmax is unnormalized exp; denominators are materialized
    partition-replicated via ones-matmuls and divided out with
    reciprocal_approx_fast + multiply on DVE (as in v1).
  - rstd values use exp(-0.5*ln(var+eps)) so the ACT engine stays on the
    natural_log_exp table set (no table thrash with the softmax Exp).
  - Batch 1's x DMA / layernorm / k,v projections / groupnorm stats are
    emitted interleaved with batch 0's chunk loop to keep PE dense.
"""

import numpy as np
import ml_dtypes

import concourse.bass as bass
import concourse.tile as tile
from concourse import bacc
from concourse import mybir
from concourse.bass_utils import run_bass_kernel_spmd

F32 = mybir.dt.float32
BF16 = mybir.dt.bfloat16
AF = mybir.ActivationFunctionType
ALU = mybir.AluOpType

B, C, H, W = 16, 512, 64, 64
HW = H * W
S, CTX = 77, 768
HEADS = 8
HD = C // HEADS  # 64
GROUPS = 32
EPS = 1e-5
NCORES = 8
BPC = B // NCORES  # 2 batches per core
P = 128
NCH = HW // 512  # 8 pixel chunks of 512
KQ = C // P      # 4 chunks of 128 for C-contraction
KC = CTX // P    # 6 chunks for CTX-contraction
GPC = GROUPS // KQ  # 8 groups per 128-channel chunk
SCALE = HD ** (-0.5)


def _emit_ctx_phase(nc, pools, consts, st, b, ctxr):
    bp, ps_qk, ps_d, ps_av = pools["bp"], pools["ps_qk"], pools["ps_d"], pools["ps_av"]
    ctx_t = bp.tile([S, CTX], F32, tag="ctx")
    nc.sync.dma_start(ctx_t, ctxr[b])
    lst = bp.tile([S, 3, 6], F32, tag="lst")
    for i in range(3):
        nc.vector.bn_stats(lst[:, i, :], ctx_t[:, i * 256:(i + 1) * 256])
    lmv = bp.tile([S, 2], F32, tag="lmv")
    nc.vector.bn_aggr(lmv, lst)
    ltmp = bp.tile([S, 1], F32, tag="ltmp")
    nc.scalar.activation(ltmp, lmv[:, 1:2], AF.Ln, bias=consts["eps77"],
                         scale=1.0)
    lrs = bp.tile([S, 1], F32, tag="lrs")
    nc.scalar.activation(lrs, ltmp, AF.Exp, bias=0.0, scale=-0.5)
    cn = bp.tile([S, CTX], BF16, tag="cn")
    nc.vector.tensor_scalar(cn, ctx_t, lmv[:, 0:1], lrs, ALU.subtract, ALU.mult)
    cnT = bp.tile([P, KC, S], BF16, tag="cnT")
    for kc in range(KC):
        pt = ps_d.tile([P, S], BF16, tag="pd")
        nc.tensor.transpose(pt, cn[:, kc * P:(kc + 1) * P],
                            consts["identb"][:S, :S])
        nc.vector.tensor_copy(cnT[:, kc, :], pt)
    # k projection -> kT [128, 4, 77] bf16 (c on partitions), bias in evac
    kT = bp.tile([P, KQ, S], BF16, tag="kT")
    for mo in range(KQ):
        pk = ps_av.tile([P, S], F32, tag="pav")
        for kc in range(KC):
            nc.tensor.matmul(pk, consts["kwT"][:, kc, mo * P:(mo + 1) * P],
                             cnT[:, kc, :], start=(kc == 0), stop=(kc == KC - 1))
        nc.scalar.activation(kT[:, mo, :], pk, AF.Identity,
                             bias=consts["kb2"][:, mo:mo + 1], scale=1.0)
    # v projection -> v_sc [77, 512] bf16 (s on partitions), NO bias (folded)
    pv = ps_qk.tile([S, C], F32, tag="pa")
    for kc in range(KC):
        nc.tensor.matmul(pv, cnT[:, kc, :], consts["vwT"][:, kc, :],
                         start=(kc == 0), stop=(kc == KC - 1))
    v_sc = bp.tile([S, C], BF16, tag="vsc")
    nc.vector.tensor_copy(v_sc, pv)
    st[b]["kT"] = kT
    st[b]["v_sc"] = v_sc


def _emit_x_dma(nc, pools, st, b, xr):
    xb = pools["xp"].tile([P, KQ, HW], BF16, tag="xb")
    for co in range(KQ):
        nc.sync.dma_start(xb[:, co, :], xr[b, :, co, :])
    st[b]["xb"] = xb
    gst = pools["bp"].tile([P, KQ, 8, 6], F32, tag="gst")
    mv_c = pools["bp"].tile([P, KQ, 2], F32, tag="mvc")
    st[b]["gst"] = gst
    st[b]["mv_c"] = mv_c


def _emit_stats_co(nc, st, b, co):
    xb, gst, mv_c = st[b]["xb"], st[b]["gst"], st[b]["mv_c"]
    for sg in range(8):
        nc.vector.bn_stats(gst[:, co, sg, :], xb[:, co, sg * 512:(sg + 1) * 512])
    nc.vector.bn_aggr(mv_c[:, co, :], gst[:, co])


def _emit_stats_combine(nc, pools, consts, st, b):
    bp, ps_d, ps_av = pools["bp"], pools["ps_d"], pools["ps_av"]
    mv_c = st[b]["mv_c"]
    # per-channel stats of the original x: mean = mean' - cb, var unchanged
    t3 = bp.tile([P, KQ, 3], F32, tag="t3")
    nc.vector.tensor_sub(t3[:, :, 0], mv_c[:, :, 0], consts["cb"])
    nc.vector.tensor_copy(t3[:, :, 1], mv_c[:, :, 1])
    nc.vector.tensor_mul(t3[:, :, 2], t3[:, :, 0], t3[:, :, 0])
    pg = ps_d.tile([GPC, KQ * 3], F32, tag="pd")
    nc.tensor.matmul(pg, consts["ind1"],
                     t3.rearrange("p a b -> p (a b)"), start=True, stop=True)
    g_sb = bp.tile([GPC, KQ, 3], F32, tag="gsb")
    nc.vector.tensor_copy(g_sb.rearrange("p a b -> p (a b)"), pg)
    stats2 = bp.tile([GPC, 2, KQ], F32, tag="st2")
    nc.vector.tensor_copy(stats2[:, 0, :], g_sb[:, :, 0])
    vt = bp.tile([GPC, KQ], F32, tag="vt")
    nc.vector.tensor_add(vt, g_sb[:, :, 1], g_sb[:, :, 2])
    m2 = bp.tile([GPC, KQ], F32, tag="m2")
    nc.vector.tensor_mul(m2, g_sb[:, :, 0], g_sb[:, :, 0])
    nc.vector.tensor_sub(vt, vt, m2)
    # rstd = exp(-0.5 * ln(var + eps)) -- stays on the exp/ln table set
    vln = bp.tile([GPC, KQ], F32, tag="vln")
    nc.scalar.activation(vln, vt, AF.Ln, bias=consts["eps8"], scale=1.0)
    nc.scalar.activation(stats2[:, 1, :], vln, AF.Exp, bias=0.0, scale=-0.5)
    pbc = ps_av.tile([P, 2 * KQ], F32, tag="pav")
    nc.tensor.matmul(pbc, consts["ind2"],
                     stats2.rearrange("p a b -> p (a b)"), start=True, stop=True)
    sbc = bp.tile([P, 2, KQ], F32, tag="sbc")
    nc.vector.tensor_copy(sbc.rearrange("p a b -> p (a b)"), pbc)
    scale_c = bp.tile([P, KQ], F32, tag="scl")
    nc.vector.tensor_mul(scale_c, sbc[:, 1, :], consts["gnw"])
    # shift2 = gnb - (mean_bc + cb) * scale   (xn = xh*scale + shift2)
    tmpm = bp.tile([P, KQ], F32, tag="tmpm")
    nc.vector.tensor_add(tmpm, sbc[:, 0, :], consts["cb"])
    nc.vector.tensor_mul(tmpm, tmpm, scale_c)
    shift2 = bp.tile([P, KQ], F32, tag="sh2")
    nc.vector.tensor_sub(shift2, consts["gnb"], tmpm)
    sh2b = bp.tile([P, KQ], BF16, tag="sh2b")
    nc.vector.tensor_copy(sh2b, shift2)
    # scaled q weights: qwT_s[c, :] = qwT[c, :] * scale[c]
    qwT_s = bp.tile([P, KQ, C], BF16, tag="qws")
    for co in range(KQ):
        nc.vector.tensor_scalar_mul(qwT_s[:, co, :], consts["qwT"][:, co, :],
                                    scale_c[:, co:co + 1])
    # per-output-channel q bias: qt[o] = sum_c qw[o,c]*shift2[c] + qb[o]
    pqt = ps_d.tile([P, KQ], F32, tag="pd")
    for mo in range(KQ):
        for kc in range(KQ):
            nc.tensor.matmul(pqt[:, mo:mo + 1],
                             consts["qwT"][:, kc, mo * P:(mo + 1) * P],
                             sh2b[:, kc:kc + 1], start=(kc == 0),
                             stop=(kc == KQ - 1))
    qt = bp.tile([P, KQ], F32, tag="qt")
    nc.vector.tensor_add(qt, pqt, consts["qb"])
    st[b]["qwT_s"] = qwT_s
    st[b]["qt"] = qt


def _emit_chunk(nc, pools, consts, st, b, n, outr):
    ps_mm, ps_qk, ps_d, ps_av = (pools["ps_mm"], pools["ps_qk"],
                                 pools["ps_d"], pools["ps_av"])
    xb, kT, v_sc = st[b]["xb"], st[b]["kT"], st[b]["v_sc"]
    qwT_s, qt = st[b]["qwT_s"], st[b]["qt"]
    nsl = slice(n * 512, (n + 1) * 512)

    qT = pools["qp"].tile([P, KQ, 512], BF16, tag="qT")
    for mo in range(KQ):
        pq = ps_mm.tile([P, 512], F32, tag="pmm")
        for kc in range(KQ):
            nc.tensor.matmul(pq, qwT_s[:, kc, mo * P:(mo + 1) * P],
                             xb[:, kc, nsl], start=(kc == 0),
                             stop=(kc == KQ - 1))
        nc.any.tensor_scalar(qT[:, mo, :], pq, qt[:, mo:mo + 1], None, ALU.add)

    outT = pools["op"].tile([P, KQ, 512], BF16, tag="outT")
    for co in range(KQ):
        pa = ps_qk.tile([S, 2, 512], F32, tag="pa")
        nc.tensor.matmul(pa[:, 0, :], kT[0:HD, co, :], qT[0:HD, co, :],
                         start=True, stop=True, tile_position=(0, 0))
        nc.tensor.matmul(pa[:, 1, :], kT[HD:P, co, :], qT[HD:P, co, :],
                         start=True, stop=True, tile_position=(64, 0))
        ex = pools["expp"].tile([S, 2, 512], BF16, tag="ex")
        nc.scalar.activation(ex, pa, AF.Exp, scale=SCALE)
        pd = ps_d.tile([P, 512], F32, tag="pd")
        nc.tensor.matmul(pd[0:HD, :], consts["ones77"], ex[:, 0, :],
                         start=True, stop=True, tile_position=(0, 0))
        nc.tensor.matmul(pd[HD:P, :], consts["ones77"], ex[:, 1, :],
                         start=True, stop=True, tile_position=(0, 64))
        rc = pools["rcp"].tile([P, 512], F32, tag="rc")
        nc.vector.reciprocal_approx_fast(out=rc, in_=pd)
        pav = ps_av.tile([P, 512], F32, tag="pav")
        h0, h1 = 2 * co, 2 * co + 1
        nc.tensor.matmul(pav[0:HD, :], v_sc[:, h0 * HD:(h0 + 1) * HD],
                         ex[:, 0, :], start=True, stop=True,
                         tile_position=(0, 0))
        nc.tensor.matmul(pav[HD:P, :], v_sc[:, h1 * HD:(h1 + 1) * HD],
                         ex[:, 1, :], start=True, stop=True,
                         tile_position=(0, 64))
        nc.vector.tensor_mul(outT[:, co, :], pav, rc)

    # out projection with the residual folded in as an identity matmul
    for mo in range(KQ):
        po = ps_mm.tile([P, 512], F32, tag="pmm")
        for kc in range(KQ):
            nc.tensor.matmul(po, consts["owT"][:, kc, mo * P:(mo + 1) * P],
                             outT[:, kc, :], start=(kc == 0), stop=False)
        nc.tensor.matmul(po, consts["identb"], xb[:, mo, nsl],
                         start=False, stop=True)
        fin = pools["finp"].tile([P, 512], F32, tag="fin")
        nc.any.tensor_copy(fin, po)
        nc.sync.dma_start(outr[b, :, mo, nsl], fin)


def build_nc():
    nc = bacc.Bacc()

    xh = nc.dram_tensor("xh", [BPC, C, HW], BF16, kind="ExternalInput")
    ctx_in = nc.dram_tensor("ctx", [BPC, S, CTX], F32, kind="ExternalInput")
    qwT = nc.dram_tensor("qwT", [C, C], BF16, kind="ExternalInput")
    kwT = nc.dram_tensor("kwT", [CTX, C], BF16, kind="ExternalInput")
    vwT = nc.dram_tensor("vwT", [CTX, C], BF16, kind="ExternalInput")
    owT = nc.dram_tensor("owT", [C, C], BF16, kind="ExternalInput")
    qb = nc.dram_tensor("qb", [C], F32, kind="ExternalInput")
    kb2 = nc.dram_tensor("kb2", [C], F32, kind="ExternalInput")
    cb = nc.dram_tensor("cb", [C], F32, kind="ExternalInput")
    gnw = nc.dram_tensor("gnw", [C], F32, kind="ExternalInput")
    gnb = nc.dram_tensor("gnb", [C], F32, kind="ExternalInput")
    identb = nc.dram_tensor("identb", [P, P], BF16, kind="ExternalInput")
    ones77 = nc.dram_tensor("ones77", [S, HD], BF16, kind="ExternalInput")
    ind1 = nc.dram_tensor("ind1", [P, GPC], F32, kind="ExternalInput")
    ind2 = nc.dram_tensor("ind2", [GPC, P], F32, kind="ExternalInput")
    out = nc.dram_tensor("out", [BPC, C, HW], F32, kind="ExternalOutput")

    xr = xh[:].rearrange("b (co p) hw -> b p co hw", p=P)
    ctxr = ctx_in[:]
    outr = out[:].rearrange("b (co p) hw -> b p co hw", p=P)

    with tile.TileContext(nc) as tc:
        with (
            tc.tile_pool(name="singles", bufs=1) as singles,
            tc.tile_pool(name="xp", bufs=2) as x_pool,
            tc.tile_pool(name="bp", bufs=2) as bp,
            tc.tile_pool(name="qp", bufs=2) as q_pool,
            tc.tile_pool(name="op", bufs=2) as o_pool,
            tc.tile_pool(name="expp", bufs=3) as exp_pool,
            tc.tile_pool(name="rcp", bufs=2) as rc_pool,
            tc.tile_pool(name="finp", bufs=3) as fin_pool,
            tc.tile_pool(name="ps_mm", bufs=2, space="PSUM") as ps_mm,
            tc.tile_pool(name="ps_qk", bufs=2, space="PSUM") as ps_qk,
            tc.tile_pool(name="ps_d", bufs=1, space="PSUM") as ps_d,
            tc.tile_pool(name="ps_av", bufs=1, space="PSUM") as ps_av,
        ):
            pools = {"xp": x_pool, "bp": bp, "qp": q_pool, "op": o_pool,
                     "expp": exp_pool, "rcp": rc_pool, "finp": fin_pool,
                     "ps_mm": ps_mm, "ps_qk": ps_qk, "ps_d": ps_d,
                     "ps_av": ps_av}
            consts = {}
            t = singles.tile([S, HD], BF16, tag="ones77")
            nc.sync.dma_start(t, ones77[:])
            consts["ones77"] = t
            t = singles.tile([P, P], BF16, tag="identb")
            nc.sync.dma_start(t, identb[:])
            consts["identb"] = t
            t = singles.tile([P, KC, C], BF16, tag="kwT")
            nc.sync.dma_start(t, kwT[:].rearrange("(ko kp) o -> kp ko o", kp=P))
            consts["kwT"] = t
            t = singles.tile([P, KC, C], BF16, tag="vwT")
            nc.sync.dma_start(t, vwT[:].rearrange("(ko kp) o -> kp ko o", kp=P))
            consts["vwT"] = t
            t = singles.tile([P, KQ, C], BF16, tag="qwT")
            nc.sync.dma_start(t, qwT[:].rearrange("(ko kp) o -> kp ko o", kp=P))
            consts["qwT"] = t
            t = singles.tile([P, KQ, C], BF16, tag="owT")
            nc.sync.dma_start(t, owT[:].rearrange("(ko kp) o -> kp ko o", kp=P))
            consts["owT"] = t
            for name, src in (("qb", qb), ("kb2", kb2), ("cb", cb),
                              ("gnw", gnw), ("gnb", gnb)):
                t = singles.tile([P, KQ], F32, tag=name)
                nc.sync.dma_start(t, src[:].rearrange("(a p) -> p a", p=P))
                consts[name] = t
            t = singles.tile([P, GPC], F32, tag="ind1")
            nc.sync.dma_start(t, ind1[:])
            consts["ind1"] = t
            t = singles.tile([GPC, P], F32, tag="ind2")
            nc.sync.dma_start(t, ind2[:])
            consts["ind2"] = t
            t = singles.tile([S, 1], F32, tag="eps77")
            nc.vector.memset(t, EPS)
            consts["eps77"] = t
            t = singles.tile([GPC, 1], F32, tag="eps8")
            nc.vector.memset(t, EPS)
            consts["eps8"] = t

            st = {0: {}, 1: {}}
            # batch 0 prologue
            _emit_ctx_phase(nc, pools, consts, st, 0, ctxr)
            _emit_x_dma(nc, pools, st, 0, xr)
            for co in range(KQ):
                _emit_stats_co(nc, st, 0, co)
            _emit_stats_combine(nc, pools, consts, st, 0)
            # queue batch 1 inputs + ctx work early (fills PE while batch 0
            # stats land; DMA overlaps batch 0 chunks)
            _emit_x_dma(nc, pools, st, 1, xr)
            _emit_ctx_phase(nc, pools, consts, st, 1, ctxr)
            # batch 0 chunks, with batch 1 stats spread between them
            for n in range(NCH):
                _emit_chunk(nc, pools, consts, st, 0, n, outr)
                if n < KQ:
                    _emit_stats_co(nc, st, 1, n)
                elif n == KQ:
                    _emit_stats_combine(nc, pools, consts, st, 1)
            for n in range(NCH):
                _emit_chunk(nc, pools, consts, st, 1, n, outr)

    nc.finalize()
    return nc


_NC_CACHE = None


def _get_nc():
    global _NC_CACHE
    if _NC_CACHE is None:
        _NC_CACHE = build_nc()
    return _NC_CACHE


def _host_consts():
    bf = ml_dtypes.bfloat16
    ind1 = np.zeros((P, GPC), np.float32)
    for p in range(P):
        ind1[p, p // 16] = 1.0 / 16.0
    ind2 = np.zeros((GPC, P), np.float32)
    for p in range(P):
        ind2[p // 16, p] = 1.0
    return {
        "identb": np.eye(P, dtype=bf),
        "ones77": np.ones((S, HD), dtype=bf),
        "ind1": ind1,
        "ind2": ind2,
    }


def _make_in_maps(x, context, gn_w, gn_b, ln_w, ln_b, q_w, q_b, k_w, k_b,
                  v_w, v_b, out_w, out_b):
    bf = ml_dtypes.bfloat16
    f32 = np.float32
    x = np.asarray(x, f32).reshape(B, C, HW)
    context = np.ascontiguousarray(np.asarray(context, f32))
    q_w = np.asarray(q_w, f32)
    k_w = np.asarray(k_w, f32)
    v_w = np.asarray(v_w, f32)
    out_w = np.asarray(out_w, f32)
    ln_w = np.asarray(ln_w, f32)
    ln_b = np.asarray(ln_b, f32)
    kb2 = np.asarray(k_b, f32) + k_w @ ln_b
    vb2 = np.asarray(v_b, f32) + v_w @ ln_b
    cb = np.asarray(out_b, f32) + out_w @ vb2
    xh = (x + cb[None, :, None]).astype(bf)
    shared = {
        "qwT": np.ascontiguousarray(q_w.T).astype(bf),
        "kwT": np.ascontiguousarray((k_w * ln_w[None, :]).T).astype(bf),
        "vwT": np.ascontiguousarray((v_w * ln_w[None, :]).T).astype(bf),
        "owT": np.ascontiguousarray(out_w.T).astype(bf),
        "qb": np.asarray(q_b, f32),
        "kb2": kb2,
        "cb": cb,
        "gnw": np.asarray(gn_w, f32),
        "gnb": np.asarray(gn_b, f32),
        **_host_consts(),
    }
    in_maps = []
    for i in range(NCORES):
        m = dict(shared)
        m["xh"] = np.ascontiguousarray(xh[i * BPC:(i + 1) * BPC])
        m["ctx"] = np.ascontiguousarray(context[i * BPC:(i + 1) * BPC])
        in_maps.append(m)
    return in_maps


def kernel(x, context, gn_w, gn_b, ln_w, ln_b, q_w, q_b, k_w, k_b,
           v_w, v_b, out_w, out_b):
    in_maps = _make_in_maps(x, context, gn_w, gn_b, ln_w, ln_b, q_w, q_b,
                            k_w, k_b, v_w, v_b, out_w, out_b)
    nc = _get_nc()
    res = run_bass_kernel_spmd(nc, in_maps, core_ids=list(range(NCORES)))
    outs = [r["out"] for r in res.results]
    return np.concatenate(outs, axis=0).reshape(B, C, H, W)


if __name__ == "__main__":
    rng = np.random.default_rng(0)
    inputs = {
        "x": rng.standard_normal((B, C, H, W)).astype(np.float32),
        "context": rng.standard_normal((B, S, CTX)).astype(np.float32),
        "gn_w": np.ones(C, np.float32), "gn_b": np.zeros(C, np.float32),
        "ln_w": np.ones(CTX, np.float32), "ln_b": np.zeros(CTX, np.float32),
        "q_w": (rng.standard_normal((C, C)) * 0.02).astype(np.float32),
        "q_b": np.zeros(C, np.float32),
        "k_w": (rng.standard_normal((C, CTX)) * 0.02).astype(np.float32),
        "k_b": np.zeros(C, np.float32),
        "v_w": (rng.standard_normal((C, CTX)) * 0.02).astype(np.float32),
        "v_b": np.zeros(C, np.float32),
        "out_w": (rng.standard_normal((C, C)) * 0.02).astype(np.float32),
        "out_b": np.zeros(C, np.float32),
    }
    out = kernel(**inputs)
    print(out.shape, out.dtype)


# revision 24
# speedup vs baseline: 1.2305x; 1.2305x over previous
"""CrossAttentionBlock Trainium2 kernel (v2).

Shapes (hardcoded): x (16, 512, 64, 64) f32, context (16, 77, 768) f32.
Sharding: data-parallel over batch B=16 across 8 cores (2 batches/core).

Key structure (vs the naive per-op mapping):
  - All normalization affines and biases are folded into weights/biases:
      * LayerNorm w/b folded into k/v weights+biases on HOST.
      * v-bias + out-bias folded into a per-channel constant cb = out_b +
        out_w @ (v_b + v_w @ ln_b), pre-added to x on HOST (x ships as bf16
        with cb added; device subtracts cb from the measured channel means).
      * GroupNorm apply folded into the q projection per batch on DEVICE:
        qwT_s[c,o] = qwT[c,o]*scale[c] (4 small DVE ops) and the shift term
        becomes a per-output-channel bias via 16 tiny matmuls, applied for
        free in the qT psum evacuation.
      * Residual add folded into the out-projection accumulation via an
        identity-matrix matmul; the psum evacuation is a single-src copy
        that the Tile scheduler can place on either ACT or DVE.
  - Softmax is unnormalized exp; denominators are materialized
    partition-replicated via ones-matmuls and divided out with
    reciprocal_approx_fast + multiply on DVE (as in v1).
  - rstd values use exp(-0.5*ln(var+eps)) so the ACT engine stays on the
    natural_log_exp table set (no table thrash with the softmax Exp).
  - Batch 1's x DMA / layernorm / k,v projections / groupnorm stats are
    emitted interleaved with batch 0's chunk loop to keep PE dense.
"""

import numpy as np
import ml_dtypes

import concourse.bass as bass
import concourse.tile as tile
from concourse import bacc
from concourse import mybir
from concourse.bass_utils import run_bass_kernel_spmd

F32 = mybir.dt.float32
BF16 = mybir.dt.bfloat16
FP8 = mybir.dt.float8e4
DR = mybir.MatmulPerfMode.DoubleRow
WS = 64.0  # fp8 weight pre-scale (descaled in psum evacuations)
AF = mybir.ActivationFunctionType
ALU = mybir.AluOpType

B, C, H, W = 16, 512, 64, 64
HW = H * W
S, CTX = 77, 768
HEADS = 8
HD = C // HEADS  # 64
GROUPS = 32
EPS = 1e-5
NCORES = 8
BPC = B // NCORES  # 2 batches per core
P = 128
NCH = HW // 512  # 8 pixel chunks of 512
KQ = C // P      # 4 chunks of 128 for C-contraction
KC = CTX // P    # 6 chunks for CTX-contraction
GPC = GROUPS // KQ  # 8 groups per 128-channel chunk
SCALE = HD ** (-0.5)


def _emit_ctx_phase(nc, pools, consts, st, b, ctxr):
    bp, ps_qk, ps_d, ps_av = pools["bp"], pools["ps_qk"], pools["ps_d"], pools["ps_av"]
    ctx_t = bp.tile([S, CTX], F32, tag="ctx")
    nc.sync.dma_start(ctx_t, ctxr[b])
    lst = bp.tile([S, 3, 6], F32, tag="lst")
    for i in range(3):
        nc.vector.bn_stats(lst[:, i, :], ctx_t[:, i * 256:(i + 1) * 256])
    lmv = bp.tile([S, 2], F32, tag="lmv")
    nc.vector.bn_aggr(lmv, lst)
    ltmp = bp.tile([S, 1], F32, tag="ltmp")
    nc.scalar.activation(ltmp, lmv[:, 1:2], AF.Ln, bias=consts["eps77"],
                         scale=1.0)
    lrs = bp.tile([S, 1], F32, tag="lrs")
    nc.scalar.activation(lrs, ltmp, AF.Exp, bias=0.0, scale=-0.5)
    cn = bp.tile([S, CTX], BF16, tag="cn")
    nc.vector.tensor_scalar(cn, ctx_t, lmv[:, 0:1], lrs, ALU.subtract, ALU.mult)
    cnT = bp.tile([P, KC, S], BF16, tag="cnT")
    for kc in range(KC):
        pt = ps_d.tile([P, S], BF16, tag="pd")
        nc.tensor.transpose(pt, cn[:, kc * P:(kc + 1) * P],
                            consts["identb"][:S, :S])
        nc.vector.tensor_copy(cnT[:, kc, :], pt)
    # k projection -> kT [128, 4, 77] bf16 (c on partitions), bias in evac
    kT = bp.tile([P, KQ, S], BF16, tag="kT")
    for mo in range(KQ):
        pk = ps_av.tile([P, S], F32, tag="pav")
        for kc in range(KC):
            nc.tensor.matmul(pk, consts["kwT"][:, kc, mo * P:(mo + 1) * P],
                             cnT[:, kc, :], start=(kc == 0), stop=(kc == KC - 1))
        nc.scalar.activation(kT[:, mo, :], pk, AF.Identity,
                             bias=consts["kb2"][:, mo:mo + 1], scale=1.0)
    # v projection -> v_sc [77, 512] bf16 (s on partitions), NO bias (folded)
    pv = ps_qk.tile([S, C], F32, tag="pa")
    for kc in range(KC):
        nc.tensor.matmul(pv, cnT[:, kc, :], consts["vwT"][:, kc, :],
                         start=(kc == 0), stop=(kc == KC - 1))
    v_sc = bp.tile([S, C], BF16, tag="vsc")
    nc.vector.tensor_copy(v_sc, pv)
    st[b]["kT"] = kT
    st[b]["v_sc"] = v_sc


def _emit_x_dma(nc, pools, st, b, xr, x8r):
    xb = pools["xp"].tile([P, KQ, HW], BF16, tag="xb")
    for co in range(KQ):
        nc.sync.dma_start(xb[:, co, :], xr[b, :, co, :])
    x8 = pools["xp"].tile([P, KQ, HW], FP8, tag="x8")
    for co in range(KQ):
        nc.sync.dma_start(x8[:, co, :], x8r[b, :, co, :])
    st[b]["x8"] = x8
    st[b]["xb"] = xb
    gst = pools["bp"].tile([P, KQ, 8, 6], F32, tag="gst")
    mv_c = pools["bp"].tile([P, KQ, 2], F32, tag="mvc")
    st[b]["gst"] = gst
    st[b]["mv_c"] = mv_c


def _emit_stats_co(nc, st, b, co):
    xb, gst, mv_c = st[b]["xb"], st[b]["gst"], st[b]["mv_c"]
    for sg in range(8):
        nc.vector.bn_stats(gst[:, co, sg, :], xb[:, co, sg * 512:(sg + 1) * 512])
    nc.vector.bn_aggr(mv_c[:, co, :], gst[:, co])


def _emit_stats_combine(nc, pools, consts, st, b):
    bp, ps_d, ps_av = pools["bp"], pools["ps_d"], pools["ps_av"]
    mv_c = st[b]["mv_c"]
    # per-channel stats of the original x: mean = mean' - cb, var unchanged
    t3 = bp.tile([P, KQ, 3], F32, tag="t3")
    nc.vector.tensor_sub(t3[:, :, 0], mv_c[:, :, 0], consts["cb"])
    nc.vector.tensor_copy(t3[:, :, 1], mv_c[:, :, 1])
    nc.vector.tensor_mul(t3[:, :, 2], t3[:, :, 0], t3[:, :, 0])
    pg = ps_d.tile([GPC, KQ * 3], F32, tag="pd")
    nc.tensor.matmul(pg, consts["ind1"],
                     t3.rearrange("p a b -> p (a b)"), start=True, stop=True)
    g_sb = bp.tile([GPC, KQ, 3], F32, tag="gsb")
    nc.vector.tensor_copy(g_sb.rearrange("p a b -> p (a b)"), pg)
    stats2 = bp.tile([GPC, 2, KQ], F32, tag="st2")
    nc.vector.tensor_copy(stats2[:, 0, :], g_sb[:, :, 0])
    vt = bp.tile([GPC, KQ], F32, tag="vt")
    nc.vector.tensor_add(vt, g_sb[:, :, 1], g_sb[:, :, 2])
    m2 = bp.tile([GPC, KQ], F32, tag="m2")
    nc.vector.tensor_mul(m2, g_sb[:, :, 0], g_sb[:, :, 0])
    nc.vector.tensor_sub(vt, vt, m2)
    # rstd = exp(-0.5 * ln(var + eps)) -- stays on the exp/ln table set
    vln = bp.tile([GPC, KQ], F32, tag="vln")
    nc.scalar.activation(vln, vt, AF.Ln, bias=consts["eps8"], scale=1.0)
    nc.scalar.activation(stats2[:, 1, :], vln, AF.Exp, bias=0.0, scale=-0.5)
    pbc = ps_av.tile([P, 2 * KQ], F32, tag="pav")
    nc.tensor.matmul(pbc, consts["ind2"],
                     stats2.rearrange("p a b -> p (a b)"), start=True, stop=True)
    sbc = bp.tile([P, 2, KQ], F32, tag="sbc")
    nc.vector.tensor_copy(sbc.rearrange("p a b -> p (a b)"), pbc)
    scale_c = bp.tile([P, KQ], F32, tag="scl")
    nc.vector.tensor_mul(scale_c, sbc[:, 1, :], consts["gnw"])
    # shift2 = gnb - (mean_bc + cb) * scale   (xn = xh*scale + shift2)
    tmpm = bp.tile([P, KQ], F32, tag="tmpm")
    nc.vector.tensor_add(tmpm, sbc[:, 0, :], consts["cb"])
    nc.vector.tensor_mul(tmpm, tmpm, scale_c)
    shift2 = bp.tile([P, KQ], F32, tag="sh2")
    nc.vector.tensor_sub(shift2, consts["gnb"], tmpm)
    sh2b = bp.tile([P, KQ], BF16, tag="sh2b")
    nc.vector.tensor_copy(sh2b, shift2)
    # scaled fp8 q weights: qw8_s[c, :] = qw8[c, :] * scale[c]  (qw8 already
    # carries the WS pre-scale)
    qw8_s = bp.tile([P, KQ, C], FP8, tag="qws")
    for co in range(KQ):
        nc.vector.tensor_scalar_mul(qw8_s[:, co, :], consts["qw8"][:, co, :],
                                    scale_c[:, co:co + 1])
    # per-output-channel q bias: qt[o] = sum_c qw[o,c]*shift2[c] + qb[o]
    pqt = ps_d.tile([P, KQ], F32, tag="pd")
    for mo in range(KQ):
        for kc in range(KQ):
            nc.tensor.matmul(pqt[:, mo:mo + 1],
                             consts["qwT"][:, kc, mo * P:(mo + 1) * P],
                             sh2b[:, kc:kc + 1], start=(kc == 0),
                             stop=(kc == KQ - 1))
    qt = bp.tile([P, KQ], F32, tag="qt")
    nc.vector.tensor_add(qt, pqt, consts["qb"])
    st[b]["qw8_s"] = qw8_s
    st[b]["qt"] = qt


def _emit_chunk(nc, pools, consts, st, b, n, outr):
    ps_mm, ps_qk, ps_d, ps_av = (pools["ps_mm"], pools["ps_qk"],
                                 pools["ps_d"], pools["ps_av"])
    xb, x8, kT, v_sc = st[b]["xb"], st[b]["x8"], st[b]["kT"], st[b]["v_sc"]
    qw8_s, qt = st[b]["qw8_s"], st[b]["qt"]
    nsl = slice(n * 512, (n + 1) * 512)

    qT = pools["qp"].tile([P, KQ, 512], BF16, tag="qT")
    for mo in range(KQ):
        pq = ps_mm.tile([P, 512], F32, tag="pmm")
        for pr in range(2):
            nc.tensor.matmul(pq, qw8_s[:, 2 * pr:2 * pr + 2, mo * P:(mo + 1) * P],
                             x8[:, 2 * pr:2 * pr + 2, nsl], start=(pr == 0),
                             stop=(pr == 1), perf_mode=DR)
        nc.any.tensor_scalar(qT[:, mo, :], pq, 1.0 / WS, qt[:, mo:mo + 1],
                             ALU.mult, ALU.add)

    outT = pools["op"].tile([P, KQ, 512], FP8, tag="outT")
    for co in range(KQ):
        pa = ps_qk.tile([S, 2, 512], F32, tag="pa")
        nc.tensor.matmul(pa[:, 0, :], kT[0:HD, co, :], qT[0:HD, co, :],
                         start=True, stop=True, tile_position=(0, 0))
        nc.tensor.matmul(pa[:, 1, :], kT[HD:P, co, :], qT[HD:P, co, :],
                         start=True, stop=True, tile_position=(64, 0))
        ex = pools["expp"].tile([S, 2, 512], BF16, tag="ex")
        nc.scalar.activation(ex, pa, AF.Exp, scale=SCALE)
        pd = ps_d.tile([P, 512], F32, tag="pd")
        nc.tensor.matmul(pd[0:HD, :], consts["ones77"], ex[:, 0, :],
                         start=True, stop=True, tile_position=(0, 0))
        nc.tensor.matmul(pd[HD:P, :], consts["ones77"], ex[:, 1, :],
                         start=True, stop=True, tile_position=(0, 64))
        rc = pools["rcp"].tile([P, 512], F32, tag="rc")
        nc.vector.reciprocal_approx_fast(out=rc, in_=pd)
        pav = ps_av.tile([P, 512], F32, tag="pav")
        h0, h1 = 2 * co, 2 * co + 1
        nc.tensor.matmul(pav[0:HD, :], v_sc[:, h0 * HD:(h0 + 1) * HD],
                         ex[:, 0, :], start=True, stop=True,
                         tile_position=(0, 0))
        nc.tensor.matmul(pav[HD:P, :], v_sc[:, h1 * HD:(h1 + 1) * HD],
                         ex[:, 1, :], start=True, stop=True,
                         tile_position=(0, 64))
        nc.vector.tensor_mul(outT[:, co, :], pav, rc)

    # out projection (fp8 DoubleRow, weights pre-scaled by WS) with the
    # residual folded in via a WS-scaled identity matmul; evac descales.
    for mo in range(KQ):
        po = ps_mm.tile([P, 512], F32, tag="pmm")
        for pr in range(2):
            nc.tensor.matmul(po, consts["ow8"][:, 2 * pr:2 * pr + 2,
                                              mo * P:(mo + 1) * P],
                             outT[:, 2 * pr:2 * pr + 2, :], start=(pr == 0),
                             stop=False, perf_mode=DR)
        nc.tensor.matmul(po, consts["ident64"], xb[:, mo, nsl],
                         start=False, stop=True)
        fin = pools["finp"].tile([P, 512], F32, tag="fin")
        nc.any.tensor_scalar(fin, po, 1.0 / WS, None, ALU.mult)
        nc.sync.dma_start(outr[b, :, mo, nsl], fin)


def build_nc():
    nc = bacc.Bacc()

    xh = nc.dram_tensor("xh", [BPC, C, HW], BF16, kind="ExternalInput")
    xh8 = nc.dram_tensor("xh8", [BPC, C, HW], FP8, kind="ExternalInput")
    ctx_in = nc.dram_tensor("ctx", [BPC, S, CTX], F32, kind="ExternalInput")
    qwT = nc.dram_tensor("qwT", [C, C], BF16, kind="ExternalInput")
    qw8T = nc.dram_tensor("qw8T", [C, C], FP8, kind="ExternalInput")
    kwT = nc.dram_tensor("kwT", [CTX, C], BF16, kind="ExternalInput")
    vwT = nc.dram_tensor("vwT", [CTX, C], BF16, kind="ExternalInput")
    ow8T = nc.dram_tensor("ow8T", [C, C], FP8, kind="ExternalInput")
    qb = nc.dram_tensor("qb", [C], F32, kind="ExternalInput")
    kb2 = nc.dram_tensor("kb2", [C], F32, kind="ExternalInput")
    cb = nc.dram_tensor("cb", [C], F32, kind="ExternalInput")
    gnw = nc.dram_tensor("gnw", [C], F32, kind="ExternalInput")
    gnb = nc.dram_tensor("gnb", [C], F32, kind="ExternalInput")
    identb = nc.dram_tensor("identb", [P, P], BF16, kind="ExternalInput")
    ident64 = nc.dram_tensor("ident64", [P, P], BF16, kind="ExternalInput")
    ones77 = nc.dram_tensor("ones77", [S, HD], BF16, kind="ExternalInput")
    ind1 = nc.dram_tensor("ind1", [P, GPC], F32, kind="ExternalInput")
    ind2 = nc.dram_tensor("ind2", [GPC, P], F32, kind="ExternalInput")
    out = nc.dram_tensor("out", [BPC, C, HW], F32, kind="ExternalOutput")

    xr = xh[:].rearrange("b (co p) hw -> b p co hw", p=P)
    x8r = xh8[:].rearrange("b (co p) hw -> b p co hw", p=P)
    ctxr = ctx_in[:]
    outr = out[:].rearrange("b (co p) hw -> b p co hw", p=P)

    with tile.TileContext(nc) as tc:
        with (
            tc.tile_pool(name="singles", bufs=1) as singles,
            tc.tile_pool(name="xp", bufs=2) as x_pool,
            tc.tile_pool(name="bp", bufs=2) as bp,
            tc.tile_pool(name="qp", bufs=2) as q_pool,
            tc.tile_pool(name="op", bufs=2) as o_pool,
            tc.tile_pool(name="expp", bufs=3) as exp_pool,
            tc.tile_pool(name="rcp", bufs=2) as rc_pool,
            tc.tile_pool(name="finp", bufs=3) as fin_pool,
            tc.tile_pool(name="ps_mm", bufs=2, space="PSUM") as ps_mm,
            tc.tile_pool(name="ps_qk", bufs=2, space="PSUM") as ps_qk,
            tc.tile_pool(name="ps_d", bufs=1, space="PSUM") as ps_d,
            tc.tile_pool(name="ps_av", bufs=1, space="PSUM") as ps_av,
        ):
            pools = {"xp": x_pool, "bp": bp, "qp": q_pool, "op": o_pool,
                     "expp": exp_pool, "rcp": rc_pool, "finp": fin_pool,
                     "ps_mm": ps_mm, "ps_qk": ps_qk, "ps_d": ps_d,
                     "ps_av": ps_av}
            consts = {}
            t = singles.tile([S, HD], BF16, tag="ones77")
            nc.sync.dma_start(t, ones77[:])
            consts["ones77"] = t
            t = singles.tile([P, P], BF16, tag="identb")
            nc.sync.dma_start(t, identb[:])
            consts["identb"] = t
            t = singles.tile([P, P], BF16, tag="ident64")
            nc.sync.dma_start(t, ident64[:])
            consts["ident64"] = t
            t = singles.tile([P, KC, C], BF16, tag="kwT")
            nc.sync.dma_start(t, kwT[:].rearrange("(ko kp) o -> kp ko o", kp=P))
            consts["kwT"] = t
            t = singles.tile([P, KC, C], BF16, tag="vwT")
            nc.sync.dma_start(t, vwT[:].rearrange("(ko kp) o -> kp ko o", kp=P))
            consts["vwT"] = t
            t = singles.tile([P, KQ, C], BF16, tag="qwT")
            nc.sync.dma_start(t, qwT[:].rearrange("(ko kp) o -> kp ko o", kp=P))
            consts["qwT"] = t
            t = singles.tile([P, KQ, C], FP8, tag="qw8")
            nc.sync.dma_start(t, qw8T[:].rearrange("(ko kp) o -> kp ko o", kp=P))
            consts["qw8"] = t
            t = singles.tile([P, KQ, C], FP8, tag="ow8")
            nc.sync.dma_start(t, ow8T[:].rearrange("(ko kp) o -> kp ko o", kp=P))
            consts["ow8"] = t
            for name, src in (("qb", qb), ("kb2", kb2), ("cb", cb),
                              ("gnw", gnw), ("gnb", gnb)):
                t = singles.tile([P, KQ], F32, tag=name)
                nc.sync.dma_start(t, src[:].rearrange("(a p) -> p a", p=P))
                consts[name] = t
            t = singles.tile([P, GPC], F32, tag="ind1")
            nc.sync.dma_start(t, ind1[:])
            consts["ind1"] = t
            t = singles.tile([GPC, P], F32, tag="ind2")
            nc.sync.dma_start(t, ind2[:])
            consts["ind2"] = t
            t = singles.tile([S, 1], F32, tag="eps77")
            nc.vector.memset(t, EPS)
            consts["eps77"] = t
            t = singles.tile([GPC, 1], F32, tag="eps8")
            nc.vector.memset(t, EPS)
            consts["eps8"] = t

            st = {0: {}, 1: {}}
            # batch 0 prologue (x DMA queued first so stats start ASAP)
            _emit_x_dma(nc, pools, st, 0, xr, x8r)
            _emit_ctx_phase(nc, pools, consts, st, 0, ctxr)
            for co in range(KQ):
                _emit_stats_co(nc, st, 0, co)
            _emit_stats_combine(nc, pools, consts, st, 0)
            # queue batch 1 inputs + ctx work early (fills PE while batch 0
            # stats land; DMA overlaps batch 0 chunks)
            _emit_x_dma(nc, pools, st, 1, xr, x8r)
            _emit_ctx_phase(nc, pools, consts, st, 1, ctxr)
            # batch 0 chunks, with batch 1 stats spread between them
            for n in range(NCH):
                _emit_chunk(nc, pools, consts, st, 0, n, outr)
                if n < KQ:
                    _emit_stats_co(nc, st, 1, n)
                elif n == KQ:
                    _emit_stats_combine(nc, pools, consts, st, 1)
            for n in range(NCH):
                _emit_chunk(nc, pools, consts, st, 1, n, outr)

    nc.finalize()
    return nc


_NC_CACHE = None


def _get_nc():
    global _NC_CACHE
    if _NC_CACHE is None:
        _NC_CACHE = build_nc()
    return _NC_CACHE


def _host_consts():
    bf = ml_dtypes.bfloat16
    ind1 = np.zeros((P, GPC), np.float32)
    for p in range(P):
        ind1[p, p // 16] = 1.0 / 16.0
    ind2 = np.zeros((GPC, P), np.float32)
    for p in range(P):
        ind2[p // 16, p] = 1.0
    return {
        "identb": np.eye(P, dtype=bf),
        "ident64": (np.eye(P, dtype=np.float32) * WS).astype(bf),
        "ones77": np.ones((S, HD), dtype=bf),
        "ind1": ind1,
        "ind2": ind2,
    }


def _make_in_maps(x, context, gn_w, gn_b, ln_w, ln_b, q_w, q_b, k_w, k_b,
                  v_w, v_b, out_w, out_b):
    bf = ml_dtypes.bfloat16
    f32 = np.float32
    x = np.asarray(x, f32).reshape(B, C, HW)
    context = np.ascontiguousarray(np.asarray(context, f32))
    q_w = np.asarray(q_w, f32)
    k_w = np.asarray(k_w, f32)
    v_w = np.asarray(v_w, f32)
    out_w = np.asarray(out_w, f32)
    ln_w = np.asarray(ln_w, f32)
    ln_b = np.asarray(ln_b, f32)
    fp8 = ml_dtypes.float8_e4m3
    kb2 = np.asarray(k_b, f32) + k_w @ ln_b
    vb2 = np.asarray(v_b, f32) + v_w @ ln_b
    cb = np.asarray(out_b, f32) + out_w @ vb2
    xf = x + cb[None, :, None]
    xh = xf.astype(bf)
    shared = {
        "qwT": np.ascontiguousarray(q_w.T).astype(bf),
        "qw8T": np.ascontiguousarray(q_w.T * WS).astype(fp8),
        "kwT": np.ascontiguousarray((k_w * ln_w[None, :]).T).astype(bf),
        "vwT": np.ascontiguousarray((v_w * ln_w[None, :]).T).astype(bf),
        "ow8T": np.ascontiguousarray(out_w.T * WS).astype(fp8),
        "qb": np.asarray(q_b, f32),
        "kb2": kb2,
        "cb": cb,
        "gnw": np.asarray(gn_w, f32),
        "gnb": np.asarray(gn_b, f32),
        **_host_consts(),
    }
    xh8 = xf.astype(fp8)
    in_maps = []
    for i in range(NCORES):
        m = dict(shared)
        m["xh"] = np.ascontiguousarray(xh[i * BPC:(i + 1) * BPC])
        m["xh8"] = np.ascontiguousarray(xh8[i * BPC:(i + 1) * BPC])
        m["ctx"] = np.ascontiguousarray(context[i * BPC:(i + 1) * BPC])
        in_maps.append(m)
    return in_maps


def kernel(x, context, gn_w, gn_b, ln_w, ln_b, q_w, q_b, k_w, k_b,
           v_w, v_b, out_w, out_b):
    in_maps = _make_in_maps(x, context, gn_w, gn_b, ln_w, ln_b, q_w, q_b,
                            k_w, k_b, v_w, v_b, out_w, out_b)
    nc = _get_nc()
    res = run_bass_kernel_spmd(nc, in_maps, core_ids=list(range(NCORES)))
    outs = [r["out"] for r in res.results]
    return np.concatenate(outs, axis=0).reshape(B, C, H, W)


if __name__ == "__main__":
    rng = np.random.default_rng(0)
    inputs = {
        "x": rng.standard_normal((B, C, H, W)).astype(np.float32),
        "context": rng.standard_normal((B, S, CTX)).astype(np.float32),
        "gn_w": np.ones(C, np.float32), "gn_b": np.zeros(C, np.float32),
        "ln_w": np.ones(CTX, np.float32), "ln_b": np.zeros(CTX, np.float32),
        "q_w": (rng.standard_normal((C, C)) * 0.02).astype(np.float32),
        "q_b": np.zeros(C, np.float32),
        "k_w": (rng.standard_normal((C, CTX)) * 0.02).astype(np.float32),
        "k_b": np.zeros(C, np.float32),
        "v_w": (rng.standard_normal((C, CTX)) * 0.02).astype(np.float32),
        "v_b": np.zeros(C, np.float32),
        "out_w": (rng.standard_normal((C, C)) * 0.02).astype(np.float32),
        "out_b": np.zeros(C, np.float32),
    }
    out = kernel(**inputs)
    print(out.shape, out.dtype)
